# revision 55
# baseline (speedup 1.0000x reference)
"""Trainium2 Bass kernel for multi-head attention (B=2, T=2048, C=1024, H=16, DH=64).

Sharding: tensor-parallel over heads. Each of the 8 cores computes 2 heads:
q/k/v projections for its heads, attention, and a partial output projection
(its 128-column slice of the concat-head dim against its 128-row slice of Wp).
The host sums the 8 fp16 partial outputs in fp32 and adds the bias.

Numerics (validated against a float64 oracle, rel err ~9e-3 vs the 2e-2 gate):
  - projections run as fp8e4 DoubleRow matmuls on a 3-term hi/lo split:
    x = x1+x2, W*64 = W1+W2, q ~ (x1W1) + (x2W1 + x1W2), with the two cross
    terms packed into the two DoubleRow planes of a single instruction.
  - q8 is stored [128 = 2heads x 64dims, 2 planes, tokens] fp8 with an exact
    hi/lo split (2 DVE instrs per token tile); k8 is a single fp8 plane
    [128, tokens] (1 instr). The score matmul for head h uses base partition
    h*64 for both operands (validated on HW) with the k plane broadcast-read
    into both DoubleRow planes via a stride-0 AP: s = k1^T(q1+q2).
  - exp on ACT emits one bf16 tile per key chunk, scale = 0.125/64 folded
    with the fp8 evac scales; widths crop to exactly the valid queries.
  - attention@V is transposed: exp tile is the stationary operand (128-query
    slices), [v|1] bf16 is moving (65-wide) -> full 128-partition output
    utilization plus a free softmax denominator column; normalization is a
    per-partition reciprocal+mul on DVE.
  - normalized heads are re-transposed (bf16, via identity) and multiplied
    against bf16 Wp; psum evacuates to fp16 (DVE mid-stream, ACT once the
    exp stream has ended) and is DMA'd out as fp16.
  - emission is software-pipelined: deferred AV/output/projection units
    drain between score chunks under a PE budget so the PE always has
    ready work while ACT streams the exp chain.
"""

from contextlib import ExitStack

import numpy as np

B, T, C, H, DH = 2, 2048, 1024, 16, 64
NCORES = 8
HP = H // NCORES          # heads per core
M = HP * DH               # 128 = packed head dim per core
P = 128                   # partitions / contraction chunk
QT = 512                  # query/token tile (free dim)
NEG = -30000.0            # additive mask value (exp(NEG + anything small) == 0)
KC_N = C // P             # contraction chunks for projections
ESC = 0.125 / 64.0        # exp scale: 1/sqrt(dh) / (q*8 * k*8)

# Scheduling knobs (tuned against the cost-model timeline):
#   quota[n] = score chunks emitted right after b0 proj tile n
#   d0/d1/dlast = PE drain budgets (ns) between score chunk emissions
SCHED = {"quota": (6, 8, 8), "d0": 1200, "d1": 800, "dlast": 1200,
         "part2": 900, "act_tail": 0, "dinter": 1200, "eg_mod": 1000, "inter": 0, "t0split": 0}


def _build(lens, t=T, c=C):
    """Build the per-core Bass module for batch lengths `lens` (tuple of B ints)."""
    import concourse.mybir as mybir
    import concourse.tile as tile
    from concourse import bacc
    from concourse.masks import make_identity

    f32 = mybir.dt.float32
    bf16 = mybir.dt.bfloat16
    f16 = mybir.dt.float16
    fp8 = mybir.dt.float8e4
    AF = mybir.ActivationFunctionType
    PM = mybir.MatmulPerfMode
    ALU = mybir.AluOpType

    qt = min(QT, t)
    nkv = [(l + P - 1) // P for l in lens]         # valid key chunks == token blocks
    partial = [l % P != 0 for l in lens]
    crop = [n * P for n in nkv]                    # token coverage per batch
    nq = [(cr + qt - 1) // qt for cr in crop]      # query tiles per batch
    nkv_max = max(nkv)

    def tiw(b, i):
        return min(crop[b] - i * qt, qt)           # multiples of 128

    def qvw(b, i):
        # exact count of valid queries in tile i (<= tiw); >= 1 always
        return max(1, min(lens[b] - i * qt, tiw(b, i)))

    nc = bacc.Bacc("TRN2", target_bir_lowering=False, debug=False,
                   num_devices=NCORES)

    x8_d = nc.dram_tensor("x8", [P, KC_N, 2, B * t], fp8, kind="ExternalInput").ap()
    w_x_d = [nc.dram_tensor(f"w{n}x", [P, KC_N, 2, M], fp8, kind="ExternalInput").ap()
             for n in ("q", "k", "v")]
    wp_d = nc.dram_tensor("wp", [M, c], bf16, kind="ExternalInput").ap()
    km_d = nc.dram_tensor("km", [P, B], f32, kind="ExternalInput").ap()
    out_d = nc.dram_tensor("out", [B * t, c], f16, kind="ExternalOutput").ap()

    with tile.TileContext(nc) as tc, ExitStack() as ctx:
        const = ctx.enter_context(tc.tile_pool(name="const", bufs=1))
        persist = ctx.enter_context(tc.tile_pool(name="persist", bufs=1))

        identb = const.tile([P, P], bf16)
        make_identity(nc, identb[:])
        kmask = const.tile([P, B], f32)
        wp_sb = const.tile([P, c], bf16)

        # q8: [128 = 2h x 64 dims, 2 planes (q1, q2), token]; score matmuls for
        # head h address base partition h*64 on both operands.
        # k8: single plane [128, token]; broadcast-read into both DR planes.
        q8 = persist.tile([P, 2, B * t], fp8, tag="q8")
        k8 = persist.tile([P, 1, B * t], fp8, tag="k8")
        vTb = persist.tile([P, B * t], bf16, tag="vTb")
        vaug = persist.tile([P, B, nkv_max, 2 * (DH + 1)], bf16, tag="vaug")

        with tc.tile_pool(name="wpool", bufs=1) as wpool, \
             tc.tile_pool(name="xpool", bufs=SCHED.get("xbufs", 5)) as xpool, \
             tc.tile_pool(name="exps", bufs=SCHED.get("etbufs", 40)) as expp, \
             tc.tile_pool(name="aob", bufs=9) as aobp, \
             tc.tile_pool(name="aot", bufs=9) as aotp, \
             tc.tile_pool(name="stage", bufs=6) as stage, \
             tc.tile_pool(name="recp", bufs=16) as recp, \
             tc.tile_pool(name="work", bufs=2, space="PSUM") as workp, \
             tc.tile_pool(name="psc", bufs=2, space="PSUM") as pscp, \
             tc.tile_pool(name="pav", bufs=2, space="PSUM") as pavp:

            w_x = []
            for i, n in enumerate(("q", "k", "v")):
                wx = wpool.tile([P, KC_N, 2, M], fp8, tag=f"w{n}x", name=f"w{n}x")
                w_x.append(wx)
            # only the softmax-denominator "ones" columns need initialization;
            # the v columns are filled per chunk and unused chunks are never read
            ones_cols = vaug[:].rearrange("p b k (g w) -> p b k g w",
                                          g=2)[:, :, :, :, DH:DH + 1]
            nc.vector.memset(ones_cols, 1.0)

            # Warm-up: dependency-free matmuls release the PE clock gate and
            # bridge the PE to the first x arrival (an idle gap resets the
            # p-state ramp); a dummy Exp preloads the ACT table set
            nwarm = SCHED.get("warm", 22)
            warm = workp.tile([P, qt], f32, tag="work", name="warm")
            for i in range(nwarm):
                nc.tensor.matmul(warm[:, 0:P], identb[:], identb[:],
                                 start=(i == 0), stop=(i == nwarm - 1))
            dummy = const.tile([P, P], f32, name="dummy")
            nc.scalar.activation(dummy[:], identb[:], AF.Exp)

            def emit_proj_dma(b, n):
                tok0 = b * t + n * qt
                tw = tiw(b, n)
                xtile = xpool.tile([P, KC_N, 2, qt], fp8, tag="x", name="xtile")
                if b == 0 and n == 0:
                    # startup critical path: x half A first (it is the long
                    # pole), then wq+wk, half B; wv and the mask/Wp constants
                    # ride behind tile 1's x
                    half = KC_N // 2
                    nc.sync.dma_start(w_x[0][:], w_x_d[0][:])
                    nc.sync.dma_start(
                        xtile[:, 0:half, :, 0:tw],
                        x8_d[:, 0:half, :, tok0:tok0 + tw])
                    nc.sync.dma_start(w_x[1][:], w_x_d[1][:])
                    nc.sync.dma_start(
                        xtile[:, half:KC_N, :, 0:tw],
                        x8_d[:, half:KC_N, :, tok0:tok0 + tw])
                    nc.sync.dma_start(w_x[2][:], w_x_d[2][:])
                elif b == 0:
                    # still on the startup critical path: halves let the
                    # cross matmuls start before the full tile lands
                    half = KC_N // 2
                    nc.sync.dma_start(
                        xtile[:, 0:half, :, 0:tw],
                        x8_d[:, 0:half, :, tok0:tok0 + tw])
                    nc.sync.dma_start(
                        xtile[:, half:KC_N, :, 0:tw],
                        x8_d[:, half:KC_N, :, tok0:tok0 + tw])
                    if n == SCHED.get("km_n", 1):
                        nc.sync.dma_start(kmask[:], km_d[:])
                        nc.sync.dma_start(wp_sb[:], wp_d[:])
                else:
                    # prefetched far ahead: one descriptor, less HWDGE serial
                    nc.sync.dma_start(
                        xtile[:, :, :, 0:tw], x8_d[:, :, :, tok0:tok0 + tw])
                return xtile

            def emit_proj_mm_part(b, n, i, xtile, holder, part):
                # emitted in order 0,1,2 = cross half A, cross half B,
                # main term (x1@W1) + evac; cross half A needs only the
                # first half of the x tile, so the pipeline starts earlier
                tw = tiw(b, n)
                if part < 2:
                    if part == 0:
                        ps = workp.tile([P, qt], f32, tag="work", name="ps")
                        holder[i] = ps
                    else:
                        ps = holder[i]
                    k0 = part * (KC_N // 2)
                    for k in range(k0, k0 + KC_N // 2):
                        nc.tensor.matmul(
                            ps[:, 0:tw],
                            w_x[i][:, k, :, :],
                            xtile[:, k, :, 0:tw],
                            start=(k == 0), stop=False,
                            perf_mode=PM.DoubleRow)
                    return
                ps = holder[i]
                for k in range(KC_N // 2):
                    nc.tensor.matmul(
                        ps[:, 0:tw],
                        w_x[i][:, 2 * k:2 * k + 2, 1, :],
                        xtile[:, 2 * k:2 * k + 2, 0, 0:tw],
                        start=False, stop=(k == KC_N // 2 - 1),
                        perf_mode=PM.DoubleRow)
                emit_proj_evac(b, n, i, ps)

            def emit_proj_evac(b, n, i, ps):
                tok0 = b * t + n * qt
                tw = tiw(b, n)
                span = slice(tok0, tok0 + tw)
                if i == 0:      # q: plane0 = fp8(ps/8), plane1 = residual
                    if b == 0 and n == 0:
                        # ACT is idle pre-exp: plane0 there overlaps the
                        # k evac on DVE, shortening the first-score chain
                        nc.scalar.activation(q8[:, 0, span], ps[:, 0:tw],
                                             AF.Copy, scale=0.125)
                    else:
                        nc.vector.tensor_scalar(
                            q8[:, 0, span], ps[:, 0:tw], 0.125, None,
                            ALU.mult)
                    nc.vector.scalar_tensor_tensor(
                        q8[:, 1, span], ps[:, 0:tw], 0.125,
                        q8[:, 0, span], ALU.mult, ALU.subtract)
                elif i == 1:    # k: single fp8 plane
                    if b == 0 and n == 0:   # overlap q's residual on DVE
                        nc.scalar.activation(k8[:, 0, span], ps[:, 0:tw],
                                             AF.Copy, scale=0.125)
                    else:
                        nc.vector.tensor_scalar(
                            k8[:, 0, span], ps[:, 0:tw], 0.125, None,
                            ALU.mult)
                else:           # v: bf16 at true scale
                    nc.vector.tensor_scalar(
                        vTb[:, span], ps[:, 0:tw], 1.0 / 64.0, None, ALU.mult)

            def emit_proj_mm(b, n, i, xtile):
                holder = {}
                for part in range(3):
                    emit_proj_mm_part(b, n, i, xtile, holder, part)

            def emit_proj_tile(b, n):
                xtile = emit_proj_dma(b, n)
                for i in range(3):
                    emit_proj_mm(b, n, i, xtile)

            def emit_vaug_chunks(b, k0, k1):
                # up to 4 transposes share one psum tile; single strided copy
                nk = k1 - k0
                pt = workp.tile([P, qt], bf16, tag="work", name="pt")
                for k in range(k0, k1):
                    key0 = b * t + k * P
                    nc.tensor.transpose(pt[:, (k - k0) * P:(k - k0 + 1) * P],
                                        vTb[:, key0:key0 + P], identb[:])
                dst = vaug[:, b, k0:k1, :].rearrange(
                    "p n (g w) -> p n g w", g=2)[:, :, :, 0:DH]
                nc.vector.tensor_copy(
                    dst, pt[:, 0:nk * P].rearrange(
                        "p (n g w) -> p n g w", n=nk, g=2))

            def _exp_single(b, qw, sck, k, etiles, bias):
                et = expp.tile([P, HP, qt], bf16, tag="et", name="et")
                src = sck[:, :, 0:qw]
                dst = et[:, :, 0:qw]
                if bias is None:
                    nc.scalar.activation(dst, src, AF.Exp, scale=ESC)
                else:
                    nc.scalar.activation(dst, src, AF.Exp, bias=bias,
                                         scale=ESC)
                etiles.append((k, et))

            def emit_scores_chunks(b, q, k0, k1, etiles, pend):
                q0 = b * t + q * qt
                # exact crop: queries past the batch length feed rows the
                # host discards
                ew = qvw(b, q)
                for k in range(k0, k1):
                    key0 = b * t + k * P
                    sck = pscp.tile([P, HP, qt], f32, tag="sc", name="sck")
                    for h in range(HP):
                        h64 = slice(h * DH, (h + 1) * DH)
                        nc.tensor.matmul(
                            sck[:, h, 0:ew],
                            k8[h64, 0:1, key0:key0 + P].broadcast_to(
                                [DH, 2, P]),
                            q8[h64, :, q0:q0 + ew],
                            start=True, stop=True, perf_mode=PM.DoubleRow)
                    masked = partial[b] and k == nkv[b] - 1
                    _exp_single(b, ew, sck, k, etiles,
                                kmask[:, b:b + 1] if masked else None)

            endgame = [False]   # true once the exp stream is fully emitted
            eg_ctr = [0]        # endgame blocks alternate ACT/DVE evac

            def eg_use_act():
                # in the endgame, put eg_mod-1 of every eg_mod blocks on ACT
                eg_ctr[0] += 1
                return eg_ctr[0] % SCHED["eg_mod"] != 0

            def emit_avh(b, q, qb, etiles, final=False, use_act=False):
                # AV for both heads of one 128-query block + normalize.
                # Both heads share one psum bank: h0 at cols 0:65, h1 at
                # 256:321; one accumulation group (start on first, stop on
                # last) -- pending-zero bytes are zeroed on first touch.
                col0 = qb * P
                aob = aobp.tile([P, P], bf16, tag="aob", name="aob")
                pa = pavp.tile([P, qt], f32, tag="pa", name="pa")
                nkb = nkv[b]
                for h in range(HP):
                    c0 = h * 2 * P
                    for idx, (k, et) in enumerate(etiles):
                        nc.tensor.matmul(
                            pa[:, c0:c0 + DH + 1],
                            et[:, h, col0:col0 + P],
                            vaug[:, b, k, h * (DH + 1):(h + 1) * (DH + 1)],
                            start=(h == 0 and idx == 0),
                            stop=(h == HP - 1 and idx == nkb - 1))
                rec = recp.tile([P, 2], f32, tag="rec", name="rec")
                den = pa[:].rearrange("p (g w) -> p g w", g=2)[:, :, DH:DH + 1]
                nc.vector.reciprocal(rec[:], den)
                if use_act:   # ACT is idle near the stream end
                    for h in range(HP):
                        c0 = h * 2 * P
                        nc.scalar.activation(
                            aob[:, h * DH:(h + 1) * DH], pa[:, c0:c0 + DH],
                            AF.Copy, scale=rec[:, h:h + 1])
                else:
                    # one DVE op for both heads: per-(partition, head) scale
                    # broadcast along the 64 head dims via a stride-0 AP
                    nc.vector.tensor_tensor(
                        aob[:].rearrange("p (g w) -> p g w", g=2),
                        pa[:].rearrange("p (g w) -> p g w", g=2)[:, :, 0:DH],
                        rec[:].unsqueeze(2).broadcast_to([P, 2, DH]),
                        ALU.mult)
                return aob

            def emit_top(b, q, qb, aob, final, use_act=False):
                # transpose + output projection + fp16 evac/DMA for one block
                pat = workp.tile([P, qt], bf16, tag="work", name="pat")
                nc.tensor.transpose(pat[:, 0:P], aob[:], identb[:])
                aotT = aotp.tile([P, P], bf16, tag="aotT", name="aotT")
                nc.vector.tensor_copy(aotT[:], pat[:, 0:P])
                blk = q * (qt // P) + qb
                tok0 = b * t + blk * P
                st = stage.tile([P, c], f16, tag="st", name="st")
                qn = c // 2
                for nn in range(2):
                    po = workp.tile([P, qt], f32, tag="work", name="po")
                    nc.tensor.matmul(
                        po[:, 0:qn], aotT[:], wp_sb[:, nn * qn:(nn + 1) * qn],
                        start=True, stop=True)
                    # final block: one half on ACT, one on DVE -- the two
                    # evacs run in parallel on the closing critical path
                    half_act = use_act if not final else (nn == 0)
                    if half_act:
                        nc.scalar.activation(st[:, nn * qn:(nn + 1) * qn],
                                             po[:, 0:qn], AF.Copy)
                    else:
                        nc.vector.tensor_copy(st[:, nn * qn:(nn + 1) * qn],
                                              po[:, 0:qn])
                    if final:
                        nc.sync.dma_start(
                            out_d[tok0:tok0 + P, nn * qn:(nn + 1) * qn],
                            st[:, nn * qn:(nn + 1) * qn])
                if not final:
                    nc.sync.dma_start(out_d[tok0:tok0 + P, :], st[:])

            # ---- software-pipelined emission ----
            # Backlog of deferred PE work units (cost_ns, emit_fn); drained
            # in FIFO order between score chunk-pairs so the PE always has
            # work while ACT runs the exp stream.
            backlog = []   # (cost_ns, fn, kind); kind 'pre' = needed pre-b1
            prereq_hook = [None]   # fires on 'pre:k*' units during drains

            def drain(budget_ns):
                while backlog and budget_ns > 0:
                    cost, fn, kind = backlog.pop(0)
                    fn()
                    if kind.startswith("pre:k") and prereq_hook[0]:
                        prereq_hook[0](kind)
                    budget_ns -= cost

            def drain_all():
                drain(float("inf"))

            def drain_prereq(on_unit=None):
                rest = []
                for idx, (cost, fn, kind) in enumerate(backlog):
                    if kind.startswith("pre"):
                        fn()
                        if on_unit is not None:
                            on_unit(kind)
                    else:
                        rest.append((cost, fn, kind))
                backlog[:] = rest

            def push_block_units(b, q, qb, etiles, final, use_act=False):
                holder = {}

                def do_avh(b=b, q=q, qb=qb, etiles=etiles, final=final,
                           use_act=use_act):
                    holder["act"] = use_act or (endgame[0] and eg_use_act())
                    holder["aob"] = emit_avh(b, q, qb, etiles, final,
                                             holder["act"])

                def do_top(b=b, q=q, qb=qb, final=final):
                    emit_top(b, q, qb, holder["aob"], final, holder["act"])

                av_ns = int(2 * nkv[b] * (DH + 1) * 0.42) + 100
                backlog.append((av_ns, do_avh, "av"))
                backlog.append((520, do_top, "av"))

            def push_proj_units(b, n):
                # DMA issued immediately (prefetch); matmuls deferred
                xtile = emit_proj_dma(b, n)
                units = {}
                for i in range(3):
                    holder = {}
                    lst = []
                    for part in range(3):
                        kind = "pre"
                        if i == 1 and part == 2:
                            kind = f"pre:k{n}"   # k evac done marker
                        def do_mm(b=b, n=n, i=i, xtile=xtile,
                                  holder=holder, part=part):
                            emit_proj_mm_part(b, n, i, xtile, holder, part)
                        lst.append((SCHED["part2"] if part == 2 else
                                    2 * tiw(b, n) * 5 // 12, do_mm, kind))
                    units[i] = lst
                return units

            def push_vaug_units(b):
                for k0 in range(0, nkv[b], 4):
                    k1 = min(k0 + 4, nkv[b])
                    def do_v(b=b, k0=k0, k1=k1):
                        emit_vaug_chunks(b, k0, k1)
                    backlog.append(((k1 - k0) * 60, do_v, "pre"))

            # per-(b, q-tile) score streams: chunks emitted so far + etiles
            ets_map = {}
            ksc = {}

            def escore(b, q, hi, cap=None):
                lst = ets_map.setdefault((b, q), [])
                k0 = ksc.get((b, q), 0)
                hi = min(hi, nkv[b])
                if cap is not None:
                    hi = min(hi, k0 + cap)
                if k0 < hi:
                    emit_scores_chunks(b, q, k0, hi, lst, None)
                    ksc[(b, q)] = hi
                return ksc.get((b, q), 0)

            quota_t = SCHED["quota"]
            d0, d1, dlast = SCHED["d0"], SCHED["d1"], SCHED["dlast"]

            def emit_tile0_split():
                """First b0 tile by token halves: q/k projection, evacuation
                and the first two score chunks run on tokens 0:256 while
                tokens 256:512 are still in flight on the DMA."""
                tw = tiw(0, 0)
                hw_ = tw // 2
                xtile = xpool.tile([P, KC_N, 2, qt], fp8, tag="x",
                                   name="xtile")
                nc.sync.dma_start(w_x[0][:], w_x_d[0][:])
                nc.sync.dma_start(xtile[:, :, :, 0:hw_],
                                  x8_d[:, :, :, 0:hw_])
                nc.sync.dma_start(w_x[1][:], w_x_d[1][:])
                nc.sync.dma_start(xtile[:, :, :, hw_:tw],
                                  x8_d[:, :, :, hw_:tw])
                nc.sync.dma_start(w_x[2][:], w_x_d[2][:])

                def proj_half(i, ps, t0, t1):
                    for k in range(KC_N):
                        nc.tensor.matmul(
                            ps[:, t0:t1], w_x[i][:, k, :, :],
                            xtile[:, k, :, t0:t1],
                            start=(k == 0), stop=False,
                            perf_mode=PM.DoubleRow)
                    for k in range(KC_N // 2):
                        nc.tensor.matmul(
                            ps[:, t0:t1], w_x[i][:, 2 * k:2 * k + 2, 1, :],
                            xtile[:, 2 * k:2 * k + 2, 0, t0:t1],
                            start=False, stop=(k == KC_N // 2 - 1),
                            perf_mode=PM.DoubleRow)

                def evac_half(i, ps, t0, t1):
                    span = slice(t0, t1)
                    if i == 0:
                        nc.scalar.activation(q8[:, 0, span], ps[:, t0:t1],
                                             AF.Copy, scale=0.125)
                        nc.vector.scalar_tensor_tensor(
                            q8[:, 1, span], ps[:, t0:t1], 0.125,
                            q8[:, 0, span], ALU.mult, ALU.subtract)
                    else:
                        nc.scalar.activation(k8[:, 0, span], ps[:, t0:t1],
                                             AF.Copy, scale=0.125)

                ew = qvw(0, 0)
                lst = ets_map.setdefault((0, 0), [])
                scks = []

                def sc_half(t0, t1):
                    for ki, (sck, et) in enumerate(scks):
                        for h in range(HP):
                            h64 = slice(h * DH, (h + 1) * DH)
                            nc.tensor.matmul(
                                sck[:, h, t0:t1],
                                k8[h64, 0:1, ki * P:(ki + 1) * P].broadcast_to(
                                    [DH, 2, P]),
                                q8[h64, :, t0:t1],
                                start=True, stop=True,
                                perf_mode=PM.DoubleRow)
                        nc.scalar.activation(et[:, :, t0:t1],
                                             sck[:, :, t0:t1], AF.Exp,
                                             scale=ESC)

                ps_q = workp.tile([P, qt], f32, tag="work", name="ps")
                proj_half(0, ps_q, 0, hw_)
                ps_k = workp.tile([P, qt], f32, tag="work", name="ps")
                proj_half(1, ps_k, 0, hw_)
                evac_half(0, ps_q, 0, hw_)
                evac_half(1, ps_k, 0, hw_)
                for k in range(2):
                    sck = pscp.tile([P, HP, qt], f32, tag="sc", name="sck")
                    et = expp.tile([P, HP, qt], bf16, tag="et", name="et")
                    scks.append((sck, et))
                    lst.append((k, et))
                sc_half(0, hw_)
                proj_half(0, ps_q, hw_, tw)
                proj_half(1, ps_k, hw_, tw)
                evac_half(0, ps_q, hw_, tw)
                evac_half(1, ps_k, hw_, tw)
                sc_half(hw_, ew)
                ksc[(0, 0)] = 2
                return xtile

            # b0 projections first, interleaved with early b0 score chunks
            # across every q-tile whose k-coverage exists (feeds ACT early).
            # q and k parts interleave (both gate scores); v defers behind
            # the tile's score quota.
            for n in range(nq[0]):
                if n == 0 and SCHED.get("t0split", 1) and tiw(0, 0) == qt \
                        and nkv[0] >= 2 and qvw(0, 0) > qt // 2:
                    xtile = emit_tile0_split()
                    cov = min(qt // P, nkv[0])
                    quota = max(0, quota_t[0] - 2)
                    if quota > 0:
                        escore(0, 0, cov, cap=quota)
                    emit_proj_mm(0, 0, 2, xtile)   # v, full width
                    continue
                xtile = emit_proj_dma(0, n)
                hq, hk = {}, {}
                for part in range(2):
                    emit_proj_mm_part(0, n, 0, xtile, hq, part)
                    emit_proj_mm_part(0, n, 1, xtile, hk, part)
                emit_proj_mm_part(0, n, 1, xtile, hk, 2)
                emit_proj_mm_part(0, n, 0, xtile, hq, 2)
                cov = min(((n + 1) * qt) // P, nkv[0])
                quota = quota_t[min(n, len(quota_t) - 1)]
                for qi in range(min(n + 1, nq[0])):
                    if quota <= 0:
                        break
                    k0 = ksc.get((0, qi), 0)
                    took = escore(0, qi, cov, cap=quota) - k0
                    quota -= took
                emit_proj_mm(0, n, 2, xtile)
            # queue b1 projections: q/k for every tile first (they gate the
            # b1 exp stream), then v parts and the vaug builds
            push_vaug_units(0)
            b1_units = [push_proj_units(1, n) for n in
                        (range(nq[1]) if B > 1 else [])]
            for u in b1_units:
                backlog.extend(u[0])
                backlog.extend(u[1])
            for u in b1_units:
                backlog.extend(u[2])
            if B > 1:
                push_vaug_units(1)
            def on_prereq(kind):
                if not kind.startswith("pre:k"):
                    return
                # after b1 tile n's k evac lands, emit newly covered chunks
                n = int(kind[5:])
                hi = min(((n + 1) * qt) // P, nkv[1])
                escore(1, 0, hi)

            prereq_hook[0] = on_prereq

            for qi in range(nq[0]):
                while ksc.get((0, qi), 0) < nkv[0]:
                    escore(0, qi, ksc.get((0, qi), 0) + 2)
                    drain(d0)
            # finish the b1 q/k projections now (the hook streams b1 tile-0
            # scores as each tile's k coverage lands, keeping ACT fed through
            # the b0 -> b1 transition)
            drain_prereq(on_prereq)

            all_tiles = [(b, q) for b in range(B) for q in range(nq[b])]
            nblocks = sum(tiw(b, q) // P for b, q in all_tiles)
            act_cut = nblocks - SCHED["act_tail"]
            blk_idx = 0
            deferred = []
            # interleave the chunk streams of the last two tiles (same-batch)
            # so the final tile's exp chain finishes with the pack, not after
            inter = (SCHED.get("inter", 1)
                     and len(all_tiles) >= 2
                     and all_tiles[-1][0] == all_tiles[-2][0]
                     and all_tiles[-1][0] == B - 1)
            for ti, (b, q) in enumerate(all_tiles):
                last_tile = ti == len(all_tiles) - 1
                if inter and ti == len(all_tiles) - 2:
                    bl, ql = all_tiles[-1]
                    for k in range(ksc.get((b, q), 0), nkv[b]):
                        escore(b, q, k + 1)
                        escore(bl, ql, k + 1)
                        drain(SCHED["dinter"])
                elif last_tile:
                    # final tile: pairs per drain -- its exp chain closes the
                    # run, so halve the drain-induced spacing; the last pairs
                    # go drain-free so no deferred block work sits ahead of
                    # the closing scores in the PE queue
                    nofill = SCHED.get("nofill", 4)
                    while ksc.get((b, q), 0) < nkv[b]:
                        escore(b, q, ksc.get((b, q), 0) + 2)
                        if nkv[b] - ksc.get((b, q), 0) > nofill:
                            drain(dlast)
                else:
                    for k in range(ksc.get((b, q), 0), nkv[b]):
                        escore(b, q, k + 1)
                        drain(d1)
                ets = ets_map.setdefault((b, q), [])
                nqb = tiw(b, q) // P
                if (SCHED.get("defer2", 0) and len(all_tiles) >= 2
                        and ti == len(all_tiles) - 2):
                    # hold this tile's blocks until the final tile's scores
                    # are emitted: the closing exp chain then flows at WAR
                    # pace and these blocks run post-stream on all engines
                    deferred.append((b, q, ets, nqb))
                    blk_idx += nqb
                    continue
                if last_tile:
                    for db, dq, dets, dnqb in deferred:
                        for qb in range(dnqb):
                            push_block_units(db, dq, qb, dets, final=False)
                for qb in range(nqb):
                    push_block_units(b, q, qb, ets,
                                     final=last_tile and qb == nqb - 1,
                                     use_act=blk_idx >= act_cut)
                    blk_idx += 1
                if b == 0 and q == nq[0] - 1:
                    # b1 proj/vaug units must be emitted before b1 scores
                    # reference q8/k8/vaug; b1 tile-0 scores interleave as
                    # k-coverage lands; AV fillers stay queued
                    drain_prereq(on_prereq)
            endgame[0] = True
            if len(backlog) >= 2:
                # final block's AVH+TOP first: its DMA descriptors beat the
                # stragglers into the HWDGE queue; straggler PE work overlaps
                backlog[:] = backlog[-2:] + backlog[:-2]
            drain_all()

    nc.compile()
    return nc


_module_cache = {}


def _get_module(lens):
    key = tuple((l + P - 1) // P for l in lens) + tuple(l % P == 0 for l in lens) \
        + tuple(min(l, T) for l in lens)
    if key not in _module_cache:
        _module_cache[key] = _build(lens)
    return _module_cache[key]


def kernel(x, lengths, Wq, Wk, Wv, Wp, bp):
    import ml_dtypes
    from concourse.bass_utils import run_bass_kernel_spmd

    F8 = (ml_dtypes.float8_e4m3fn if hasattr(ml_dtypes, 'float8_e4m3fn')
          else ml_dtypes.float8_e4m3)
    BF = ml_dtypes.bfloat16

    x = np.asarray(x, dtype=np.float32)
    lens = tuple(int(np.clip(int(v), 1, T)) for v in np.asarray(lengths).reshape(-1))
    Wq = np.asarray(Wq, dtype=np.float32)
    Wk = np.asarray(Wk, dtype=np.float32)
    Wv = np.asarray(Wv, dtype=np.float32)
    Wp = np.asarray(Wp, dtype=np.float32)
    bp = np.asarray(bp, dtype=np.float32)

    nc = _get_module(lens)

    # x8: [128, kc, 2, B*T] fp8 planes (hi, lo)
    xt = np.ascontiguousarray(x.reshape(B * T, C).T)          # [C, B*T]
    xr = xt.reshape(KC_N, P, B * T).transpose(1, 0, 2)        # [P, kc, B*T]
    x1 = xr.astype(F8)
    x2 = (xr - x1.astype(np.float32)).astype(F8)
    x8 = np.stack([x1, x2], axis=2)                           # [P, kc, 2, B*T]

    km = np.zeros((P, B), dtype=np.float32)
    for b in range(B):
        pc = (lens[b] - 1) // P
        idx = pc * P + np.arange(P)
        km[:, b] = np.where(idx < lens[b], 0.0, NEG).astype(np.float32)

    def wsplit(Wfull, h0):
        # per-core [C, 128] slice, scaled x64, hi/lo fp8 split
        Wc = np.concatenate([Wfull[h0 + i] for i in range(HP)], axis=1) * 64.0
        Wr = Wc.reshape(KC_N, P, M).transpose(1, 0, 2)        # [P, kc, M]
        W1 = Wr.astype(F8)
        W2 = (Wr - W1.astype(np.float32)).astype(F8)
        wx = np.stack([W2, W1], axis=2)                       # planes (W2, W1)
        return np.ascontiguousarray(W1), np.ascontiguousarray(wx)

    in_maps = []
    for core in range(NCORES):
        h0 = core * HP
        _, wqx = wsplit(Wq, h0)
        _, wkx = wsplit(Wk, h0)
        _, wvx = wsplit(Wv, h0)
        in_maps.append({
            "x8": x8.view(np.uint8),
            "wqx": wqx.view(np.uint8),
            "wkx": wkx.view(np.uint8),
            "wvx": wvx.view(np.uint8),
            "wp": np.ascontiguousarray(
                Wp[h0 * DH:(h0 + HP) * DH, :].astype(BF)).view(np.uint16),
            "km": km,
        })

    res = run_bass_kernel_spmd(nc, in_maps, list(range(NCORES)))

    out = np.zeros((B * T, C), dtype=np.float32)
    for r in res.results:
        out += np.asarray(r["out"]).view(np.float16).astype(np.float32) \
            if r["out"].dtype != np.float16 else r["out"].astype(np.float32)
    out = out.reshape(B, T, C)
    for b in range(B):
        out[b, lens[b]:, :] = 0.0
    out += bp
    return out
